# revision 27
# baseline (speedup 1.0000x reference)
"""Multi-head causal attention with RoPE on 8 trn2 NeuronCores.

Problem (hardcoded): B=2, S=2048, D=2048, H=16, Hd=128, fp32 in/out.
  q/k/v = x @ wq/wk/wv; RoPE(q,k); causal softmax(q k^T/sqrt(Hd)) @ v; out @ wo.

Sharding: core c = 4*b + g handles batch b, heads [4g, 4g+4).
  - wq/wk/wv column-parallel (512 cols per core); wo column-parallel fed by an
    AllGather of per-core attention outputs o^T inside each batch group of 4.
  - Host-side prep: x is pre-transposed (and cast to bf16) so no PE transposes
    are needed; per-head even/odd column permutation of wq/wk makes RoPE
    partition-aligned in the transposed [head_dim, S] layout.
  - All matmul operands are bf16 (PSUM accumulation stays f32); weights and
    x^T slices are resident/streamed once — total HBM traffic ~30MB/core vs
    ~96MB for the f32 weight-restreaming variant.
  - o^T is AllGathered (bf16) in four S/4 quarters, one after each
    attention q-block, so only the last gather's latency is tail-exposed
    (covered by the first three quarters of the output projection).
  - Phase 2 is software-pipelined: dn/pv matmuls lag sc/exp by 3 kb
    iterations (PSUM-bank drain hiding), and each head-pair's
    reciprocal/mult drain is emitted inside the next pair's kb loop so the
    in-order DVE queue doesn't block the next pair's tri-adds.

Cost-model sim: 321.5 us span, PE busy 305 us (95%). Quiet-window HW
matches at ~325-360 us (shared axon device; loaded windows read higher).
"""
import math
import numpy as np
import ml_dtypes

import concourse.bass as bass
import concourse.tile as tile
from concourse import bacc, mybir
from concourse.bass_utils import run_bass_kernel_spmd

F32 = mybir.dt.float32
BF16 = mybir.dt.bfloat16
EXPF = mybir.ActivationFunctionType.Exp
ADD = mybir.AluOpType.add
MULT = mybir.AluOpType.mult

B, S, D = 2, 2048, 2048
H, HD = 16, 128
HPC = 4              # heads per core
DC = HPC * HD        # 512 d_out per core
NCHUNK = D // 128    # 16 contraction chunks
SB = 512             # s-block (projection and q-block granularity)
NSB = S // SB        # 4
SCALE = 1.0 / math.sqrt(HD)
NEG = -1.0e30

RG = [[0, 1, 2, 3], [4, 5, 6, 7]]


def build_module(trace_sim=False, phases=(1, 2, 3), repeat=1, fake_gather=False):
    nc = bacc.Bacc("TRN2", target_bir_lowering=False, debug=False, num_devices=8)

    xt = nc.dram_tensor("xt", [D, S], BF16, kind="ExternalInput").ap()
    wq = nc.dram_tensor("wq", [D, DC], BF16, kind="ExternalInput").ap()
    wk = nc.dram_tensor("wk", [D, DC], BF16, kind="ExternalInput").ap()
    wv = nc.dram_tensor("wv", [D, DC], BF16, kind="ExternalInput").ap()
    wo = nc.dram_tensor("wo", [D, DC], BF16, kind="ExternalInput").ap()
    c2 = nc.dram_tensor("c2", [128, S], F32, kind="ExternalInput").ap()
    s2n = nc.dram_tensor("s2n", [128, S], F32, kind="ExternalInput").ap()
    tri = nc.dram_tensor("tri", [128, 128], F32, kind="ExternalInput").ap()
    ones = nc.dram_tensor("ones", [128, 128], BF16, kind="ExternalInput").ap()
    y = nc.dram_tensor("y", [S, DC], F32, kind="ExternalOutput").ap()

    ot_loc = [nc.dram_tensor(f"ot_loc{i}", [DC, SB], BF16) for i in range(NSB)]
    ot_full = [nc.dram_tensor(f"ot_full{i}", [D, SB], BF16) for i in range(NSB)]

    with tile.TileContext(nc, trace_sim=trace_sim) as tc:
        with tc.tile_pool(name="consts", bufs=1) as cpool:
            ones_t = cpool.tile([128, 128], BF16)
            nc.scalar.dma_start(ones_t[:], ones[:])
            tri_t = cpool.tile([128, 128], F32)
            nc.scalar.dma_start(tri_t[:], tri[:])
            c2_t = cpool.tile([128, S], F32)
            nc.gpsimd.dma_start(c2_t[:], c2[:])
            s2n_t = cpool.tile([128, S], F32)
            nc.gpsimd.dma_start(s2n_t[:], s2n[:])
            cst = dict(ones_t=ones_t, tri_t=tri_t, c2_t=c2_t, s2n_t=s2n_t)

            for rep in range(repeat):
                with tc.tile_pool(name=f"wres{rep}", bufs=1) as wpool, \
                     tc.tile_pool(name=f"qkres{rep}", bufs=1) as qkpool, \
                     tc.tile_pool(name=f"vres{rep}", bufs=1) as vpool, \
                     tc.tile_pool(name=f"p1sb{rep}", bufs=2) as p1, \
                     tc.tile_pool(name=f"p1xt{rep}", bufs=2) as p1x, \
                     tc.tile_pool(name=f"p2sb{rep}", bufs=3) as p2, \
                     tc.tile_pool(name=f"ps{rep}", bufs=1, space="PSUM") as ps:
                    # resident weights: loaded once, used by all 4 s-blocks.
                    wq_t = [wpool.tile([128, DC], BF16, name=f"wqt{c}")
                            for c in range(NCHUNK)]
                    wk_t = [wpool.tile([128, DC], BF16, name=f"wkt{c}")
                            for c in range(NCHUNK)]
                    wv_t = [wpool.tile([128, DC], BF16, name=f"wvt{c}")
                            for c in range(NCHUNK)]
                    wo_t = [wpool.tile([128, DC], BF16, name=f"wot{c}")
                            for c in range(NCHUNK)]

                    qt_res = [qkpool.tile([128, S], BF16, name=f"qt{h}")
                              for h in range(HPC)]
                    kt_res = [qkpool.tile([128, S], BF16, name=f"kt{h}")
                              for h in range(HPC)]
                    v_t = [vpool.tile([128, DC], BF16, name=f"v{kb}")
                           for kb in range(S // 128)]

                    # DMA issue order follows need order so PE starts ~2us in:
                    # sync: [xt0/wq interleaved, wk], scalar: [consts,
                    # xt block0 odds], gpsimd: [c2/s2n, wv, wo].
                    xtj0 = [p1x.tile([128, SB], BF16, tag=f"xt{c}",
                                     name=f"xt_0_{c}") for c in range(NCHUNK)]
                    for c in range(NCHUNK):
                        if c % 2 == 0:
                            nc.sync.dma_start(xtj0[c][:],
                                              xt[c * 128:(c + 1) * 128, 0:SB])
                        else:
                            nc.scalar.dma_start(xtj0[c][:],
                                                xt[c * 128:(c + 1) * 128, 0:SB])
                        if c % 2 == 1:
                            cw = c // 2
                            nc.sync.dma_start(wq_t[cw][:],
                                              wq[cw * 128:(cw + 1) * 128, :])
                    for cw in range(NCHUNK // 2, NCHUNK):
                        nc.sync.dma_start(wq_t[cw][:],
                                          wq[cw * 128:(cw + 1) * 128, :])
                    for c in range(NCHUNK):
                        nc.sync.dma_start(wk_t[c][:], wk[c * 128:(c + 1) * 128, :])
                    for c in range(NCHUNK):
                        nc.gpsimd.dma_start(wv_t[c][:], wv[c * 128:(c + 1) * 128, :])
                    for c in range(NCHUNK):
                        nc.gpsimd.dma_start(wo_t[c][:], wo[c * 128:(c + 1) * 128, :])

                    for j in range(NSB):
                        if j == 0:
                            xtj = xtj0
                        else:
                            xtj = _emit_xt_loads(nc, j, xt, p1x)
                        _p1_block(nc, j, xtj, wq_t, wk_t, wv_t, v_t,
                                  qt_res, kt_res, p1, ps, cst)
                    if 2 in phases:
                        for j in range(NSB):
                            _p2_block(nc, j, v_t, ot_loc, qt_res, kt_res,
                                      p2, ps, cst)
                            if 3 in phases:
                                if fake_gather:
                                    # timing probe: local copy at gather size
                                    for part in range(4):
                                        nc.gpsimd.dma_start(
                                            ot_full[j][part * DC:(part + 1) * DC, :],
                                            ot_loc[j][:])
                                else:
                                    nc.gpsimd.collective_compute(
                                        "AllGather", mybir.AluOpType.bypass,
                                        replica_groups=RG,
                                        ins=[ot_loc[j][:]], outs=[ot_full[j][:]])
                    if 2 not in phases:
                        for h in range(HPC):
                            nc.sync.dma_start(
                                ot_loc[0][h * 128:(h + 1) * 128, :],
                                qt_res[h][:, 0:SB])
                            nc.sync.dma_start(
                                ot_loc[1][h * 128:(h + 1) * 128, :],
                                kt_res[h][:, 0:SB])
                            nc.sync.dma_start(
                                ot_loc[1][h * 128:(h + 1) * 128, 0:DC],
                                v_t[h][:])

                    if 2 in phases and 3 in phases:
                        _phase3(nc, tc, rep, ot_full, wo_t, y, p2, ps)

    nc.compile()
    return nc


def _emit_xt_loads(nc, j, xt, p1x):
    """Stream x^T chunk slices for s-block j (bf16, 16 x [128, 512])."""
    s0 = j * SB
    xtj = [p1x.tile([128, SB], BF16, tag=f"xt{c}", name=f"xt_{j}_{c}")
           for c in range(NCHUNK)]
    for c in range(NCHUNK):
        q = nc.scalar if c % 2 else nc.sync
        q.dma_start(xtj[c][:], xt[c * 128:(c + 1) * 128, s0:s0 + SB])
    return xtj


def _p1_block(nc, j, xtj, wq_t, wk_t, wv_t, v_t, qt_res, kt_res, p1, ps, cst):
    """Projection + RoPE for s-block j: qt/kt slices [j*SB,(j+1)*SB), v blocks."""
    s0 = j * SB
    # q-pass then k-pass: chunk-outer, 4 held accumulators
    for (w_t, res_list, wtag) in ((wq_t, qt_res, "wq"), (wk_t, kt_res, "wk")):
        prj = [ps.tile([128, SB], F32, tag=f"acc{wtag}{h}", bufs=1,
                       name=f"prj{wtag}{j}_{h}") for h in range(HPC)]
        for c in range(NCHUNK):
            for h in range(HPC):
                nc.tensor.matmul(
                    prj[h][:], w_t[c][:, h * 128:(h + 1) * 128],
                    xtj[c][:],
                    start=(c == 0), stop=(c == NCHUNK - 1))
        for h in range(HPC):
            raw = p1.tile([128, SB], F32, tag="rraw", bufs=1)
            nc.scalar.copy(raw[:], prj[h][:])
            swp = p1.tile([128, SB], F32, tag="rswp", bufs=1)
            nc.gpsimd.dma_start(swp[0:64, :], raw[64:128, :])
            nc.gpsimd.dma_start(swp[64:128, :], raw[0:64, :])
            t1 = p1.tile([128, SB], F32, tag="rt1", bufs=1)
            nc.vector.tensor_tensor(
                t1[:], raw[:], cst["c2_t"][:, s0:s0 + SB], op=MULT)
            t2 = p1.tile([128, SB], F32, tag="rt2", bufs=1)
            nc.vector.tensor_tensor(
                t2[:], swp[:], cst["s2n_t"][:, s0:s0 + SB], op=MULT)
            nc.vector.tensor_tensor(
                res_list[h][:, s0:s0 + SB], t1[:], t2[:], op=ADD)

    # v-pass (natural layout), chunk-outer, into resident v_t
    vps = [ps.tile([128, DC], F32, tag=f"accwq{ss}", bufs=1,
                   name=f"vps{j}_{ss}") for ss in range(SB // 128)]
    for c in range(NCHUNK):
        for ss in range(SB // 128):
            nc.tensor.matmul(
                vps[ss][:], xtj[c][:, ss * 128:(ss + 1) * 128],
                wv_t[c][:], start=(c == 0), stop=(c == NCHUNK - 1))
    for ss in range(SB // 128):
        nc.scalar.copy(v_t[j * 4 + ss][:], vps[ss][:])


def _p2_block(nc, j, v_t, ot_loc, qt_res, kt_res, p2, ps, cst):
    """Causal attention for q-block j (keys/values blocks 0..4j+3)."""
    q0 = j * SB
    nkb = 4 * (j + 1)
    # the reciprocal/mult drain of pair hp is emitted after pair hp+1's
    # first two kb iterations, so hp+1's tri-adds aren't stuck behind it
    # in the in-order DVE queue (they gate exp -> PSUM banks -> PE).
    prev_drain = None
    for hp in range(HPC // 2):
        heads = (2 * hp, 2 * hp + 1)
        pv = {h: ps.tile([128, SB], F32, tag=f"accwk{h % 2}",
                         name=f"pv{j}_{h}") for h in heads}
        dn = {h: ps.tile([128, SB], F32, tag=f"accwk{2 + h % 2}",
                         name=f"dn{j}_{h}") for h in heads}
        # dn/pv emission lags sc/exp by LAG kb iterations so the in-order
        # PE has score matmuls queued ahead of the pv/dn PSUM-bank wait
        # (banks drain through the previous pair's reciprocal/mult chain).
        LAG = 3
        pend = []

        def flush_one():
            h, kb, lo, ep = pend.pop(0)
            nc.tensor.matmul(
                dn[h][:, lo:], cst["ones_t"][:], ep[:, lo:],
                start=(kb == 0), stop=(kb == nkb - 1),
                skip_group_check=True)
            nc.tensor.matmul(
                pv[h][:, lo:], v_t[kb][:, h * 128:(h + 1) * 128],
                ep[:, lo:],
                start=(kb == 0), stop=(kb == nkb - 1),
                skip_group_check=True)

        for kb in range(nkb):
            r = kb - 4 * j
            lo = 0 if r < 0 else r * 128
            for h in heads:
                sc = ps.tile([128, SB], F32, tag=f"accwq{(kb * 2 + h) % 4}",
                             name=f"sc{j}_{h}_{kb}")
                nc.tensor.matmul(
                    sc[:, lo:], kt_res[h][:, kb * 128:(kb + 1) * 128],
                    qt_res[h][:, q0 + lo:q0 + SB],
                    start=True, stop=True)
                if r >= 0:
                    nc.vector.tensor_tensor(
                        sc[:, r * 128:(r + 1) * 128],
                        sc[:, r * 128:(r + 1) * 128], cst["tri_t"][:], op=ADD)
                ep = p2.tile([128, SB], BF16, tag="ep", bufs=10)
                nc.scalar.activation(ep[:, lo:], sc[:, lo:], EXPF, scale=SCALE)
                pend.append((h, kb, lo, ep))
            if kb == 1 and prev_drain is not None:
                prev_drain()
                prev_drain = None
            while len(pend) > 2 * LAG:
                flush_one()
        if prev_drain is not None:
            prev_drain()
            prev_drain = None
        while pend:
            flush_one()

        def mk_drain(pv=pv, dn=dn, heads=heads):
            def drain():
                for h in heads:
                    rec = p2.tile([128, SB], F32, tag="rec", bufs=2,
                                  name=f"rec{j}_{h}")
                    nc.vector.reciprocal(rec[:], dn[h][:])
                    ot = p2.tile([128, SB], BF16, tag="ot", bufs=2,
                                 name=f"ot{j}_{h}")
                    nc.vector.tensor_tensor(ot[:], pv[h][:], rec[:], op=MULT)
                    nc.sync.dma_start(
                        ot_loc[j][h * 128:(h + 1) * 128, :], ot[:])
            return drain

        prev_drain = mk_drain()
    prev_drain()


def _phase3(nc, tc, rep, ot_full, wo_t, y, p3, ps):
    """Column-parallel output projection from gathered o^T halves."""
    for sq in range(4):
        o0 = sq * 512
        otf = [p3.tile([128, 512], BF16, tag=f"otf{c}", bufs=1,
                       name=f"otf{c}_{sq}") for c in range(NCHUNK)]
        for c in range(NCHUNK):
            q = nc.scalar if c % 2 else nc.sync
            q.dma_start(
                otf[c][:],
                ot_full[sq][c * 128:(c + 1) * 128, :])
        for ss in range(4):
            yps = ps.tile([128, DC], F32, tag=f"accwq{ss % 2}", name=f"yps{sq}_{ss}")
            for c in range(NCHUNK):
                nc.tensor.matmul(
                    yps[:], otf[c][:, ss * 128:(ss + 1) * 128],
                    wo_t[c][:], start=(c == 0), stop=(c == NCHUNK - 1))
            ysb = p3.tile([128, DC], F32, tag="ysb", bufs=2)
            nc.scalar.copy(ysb[:], yps[:])
            nc.sync.dma_start(
                y[o0 + ss * 128:o0 + (ss + 1) * 128, :], ysb[:])


_PERM = np.concatenate([np.arange(0, 128, 2), np.arange(1, 128, 2)])
_BF16 = ml_dtypes.bfloat16


def make_in_maps(x, wq, wk, wv, wo, freqs_cos, freqs_sin):
    """Host-side sharding/prep. Returns list of 8 per-core input dicts."""
    cosT = np.ascontiguousarray(freqs_cos.T.astype(np.float32))   # [64, S]
    sinT = np.ascontiguousarray(freqs_sin.T.astype(np.float32))
    c2 = np.concatenate([cosT, cosT], axis=0)                     # [128, S]
    s2n = np.concatenate([-sinT, sinT], axis=0)
    tri = np.where(np.arange(128)[None, :] >= np.arange(128)[:, None],
                   0.0, NEG).astype(np.float32)                   # [k, q]
    ones = np.ones((128, 128), dtype=_BF16)

    xts = [np.ascontiguousarray(x[b].T).astype(_BF16) for b in range(B)]
    in_maps = []
    for c in range(8):
        b, g = divmod(c, 4)
        cols = slice(g * DC, (g + 1) * DC)
        wq_c = np.ascontiguousarray(wq[:, cols]).copy()
        wk_c = np.ascontiguousarray(wk[:, cols]).copy()
        for h in range(HPC):
            blk = slice(h * 128, (h + 1) * 128)
            wq_c[:, blk] = wq_c[:, blk][:, _PERM]
            wk_c[:, blk] = wk_c[:, blk][:, _PERM]
        in_maps.append({
            "xt": xts[b],
            "wq": wq_c.astype(_BF16),
            "wk": wk_c.astype(_BF16),
            "wv": np.ascontiguousarray(wv[:, cols]).astype(_BF16),
            "wo": np.ascontiguousarray(wo[:, cols]).astype(_BF16),
            "c2": c2, "s2n": s2n, "tri": tri, "ones": ones,
        })
    return in_maps


def assemble(results):
    """Concatenate per-core column outputs into [B, S, D]."""
    out = np.empty((B, S, D), dtype=np.float32)
    for c in range(8):
        b, g = divmod(c, 4)
        out[b][:, g * DC:(g + 1) * DC] = results[c]["y"]
    return out


_NC = None


def kernel(x, wq, wk, wv, wo, freqs_cos, freqs_sin):
    global _NC
    x = np.asarray(x); wq = np.asarray(wq); wk = np.asarray(wk)
    wv = np.asarray(wv); wo = np.asarray(wo)
    freqs_cos = np.asarray(freqs_cos); freqs_sin = np.asarray(freqs_sin)
    if _NC is None:
        _NC = build_module()
    in_maps = make_in_maps(x, wq, wk, wv, wo, freqs_cos, freqs_sin)
    res = run_bass_kernel_spmd(_NC, in_maps, core_ids=list(range(8)))
    return assemble(res.results)


# revision 28
# speedup vs baseline: 1.0633x; 1.0633x over previous
"""Multi-head causal attention with RoPE on 8 trn2 NeuronCores.

Problem (hardcoded): B=2, S=2048, D=2048, H=16, Hd=128, fp32 in/out.
  q/k/v = x @ wq/wk/wv; RoPE(q,k); causal softmax(q k^T/sqrt(Hd)) @ v; out @ wo.

Sharding: core c = 4*b + g handles batch b, heads [4g, 4g+4).
  - wq/wk/wv column-parallel (512 cols per core); wo column-parallel fed by an
    AllGather of per-core attention outputs o^T inside each batch group of 4.
  - Host-side prep: x is pre-transposed (and cast to bf16) so no PE transposes
    are needed; per-head even/odd column permutation of wq/wk makes RoPE
    partition-aligned in the transposed [head_dim, S] layout.
  - All matmul operands are bf16 (PSUM accumulation stays f32); weights and
    x^T slices are resident/streamed once — total HBM traffic ~30MB/core vs
    ~96MB for the f32 weight-restreaming variant.
  - o^T is AllGathered (bf16) in four S/4 quarters, one after each
    attention q-block, so only the last gather's latency is tail-exposed
    (covered by the first three quarters of the output projection).
  - Phase 2 is software-pipelined: dn/pv matmuls lag sc/exp by 3 kb
    iterations (PSUM-bank drain hiding), and each head-pair's
    reciprocal/mult drain is emitted inside the next pair's kb loop so the
    in-order DVE queue doesn't block the next pair's tri-adds.

Cost-model sim: 321.5 us span, PE busy 305 us (95%). Quiet-window HW
matches at ~325-360 us (shared axon device; loaded windows read higher).
"""
import math
import numpy as np
import ml_dtypes

import concourse.bass as bass
import concourse.tile as tile
from concourse import bacc, mybir
from concourse.bass_utils import run_bass_kernel_spmd

F32 = mybir.dt.float32
BF16 = mybir.dt.bfloat16
EXPF = mybir.ActivationFunctionType.Exp
ADD = mybir.AluOpType.add
MULT = mybir.AluOpType.mult

B, S, D = 2, 2048, 2048
H, HD = 16, 128
HPC = 4              # heads per core
DC = HPC * HD        # 512 d_out per core
NCHUNK = D // 128    # 16 contraction chunks
SB = 512             # s-block (projection and q-block granularity)
NSB = S // SB        # 4
SCALE = 1.0 / math.sqrt(HD)
NEG = -1.0e30

RG = [[0, 1, 2, 3], [4, 5, 6, 7]]


def build_module(trace_sim=False, phases=(1, 2, 3), repeat=1, fake_gather=False):
    nc = bacc.Bacc("TRN2", target_bir_lowering=False, debug=False, num_devices=8)

    xt = nc.dram_tensor("xt", [D, S], BF16, kind="ExternalInput").ap()
    wq = nc.dram_tensor("wq", [D, DC], BF16, kind="ExternalInput").ap()
    wk = nc.dram_tensor("wk", [D, DC], BF16, kind="ExternalInput").ap()
    wv = nc.dram_tensor("wv", [D, DC], BF16, kind="ExternalInput").ap()
    wo = nc.dram_tensor("wo", [D, DC], BF16, kind="ExternalInput").ap()
    c2 = nc.dram_tensor("c2", [128, S], BF16, kind="ExternalInput").ap()
    s2n = nc.dram_tensor("s2n", [128, S], BF16, kind="ExternalInput").ap()
    tri = nc.dram_tensor("tri", [128, 128], F32, kind="ExternalInput").ap()
    ones = nc.dram_tensor("ones", [128, 128], BF16, kind="ExternalInput").ap()
    y = nc.dram_tensor("y", [S, DC], F32, kind="ExternalOutput").ap()

    ot_loc = [nc.dram_tensor(f"ot_loc{i}", [DC, SB], BF16) for i in range(NSB)]
    ot_full = [nc.dram_tensor(f"ot_full{i}", [D, SB], BF16) for i in range(NSB)]

    with tile.TileContext(nc, trace_sim=trace_sim) as tc:
        with tc.tile_pool(name="consts", bufs=1) as cpool:
            ones_t = cpool.tile([128, 128], BF16)
            nc.scalar.dma_start(ones_t[:], ones[:])
            tri_t = cpool.tile([128, 128], F32)
            nc.scalar.dma_start(tri_t[:], tri[:])
            c2_t = cpool.tile([128, S], BF16)
            nc.gpsimd.dma_start(c2_t[:], c2[:])
            s2n_t = cpool.tile([128, S], BF16)
            nc.gpsimd.dma_start(s2n_t[:], s2n[:])
            cst = dict(ones_t=ones_t, tri_t=tri_t, c2_t=c2_t, s2n_t=s2n_t)

            for rep in range(repeat):
                with tc.tile_pool(name=f"wres{rep}", bufs=1) as wpool, \
                     tc.tile_pool(name=f"qkres{rep}", bufs=1) as qkpool, \
                     tc.tile_pool(name=f"vres{rep}", bufs=1) as vpool, \
                     tc.tile_pool(name=f"p1sb{rep}", bufs=2) as p1, \
                     tc.tile_pool(name=f"p1xt{rep}", bufs=2) as p1x, \
                     tc.tile_pool(name=f"p2sb{rep}", bufs=3) as p2, \
                     tc.tile_pool(name=f"ps{rep}", bufs=1, space="PSUM") as ps:
                    # resident weights: loaded once, used by all 4 s-blocks.
                    wq_t = [wpool.tile([128, DC], BF16, name=f"wqt{c}")
                            for c in range(NCHUNK)]
                    wk_t = [wpool.tile([128, DC], BF16, name=f"wkt{c}")
                            for c in range(NCHUNK)]
                    wv_t = [wpool.tile([128, DC], BF16, name=f"wvt{c}")
                            for c in range(NCHUNK)]
                    wo_t = [wpool.tile([128, DC], BF16, name=f"wot{c}")
                            for c in range(NCHUNK)]

                    qt_res = [qkpool.tile([128, S], BF16, name=f"qt{h}")
                              for h in range(HPC)]
                    kt_res = [qkpool.tile([128, S], BF16, name=f"kt{h}")
                              for h in range(HPC)]
                    v_t = [vpool.tile([128, DC], BF16, name=f"v{kb}")
                           for kb in range(S // 128)]

                    # DMA issue order follows need order so PE starts ~2us in:
                    # sync: [xt0/wq interleaved, wk], scalar: [consts,
                    # xt block0 odds], gpsimd: [c2/s2n, wv, wo].
                    xtj0 = [p1x.tile([128, SB], BF16, tag=f"xt{c}",
                                     name=f"xt_0_{c}") for c in range(NCHUNK)]
                    for c in range(NCHUNK):
                        if c % 2 == 0:
                            nc.sync.dma_start(xtj0[c][:],
                                              xt[c * 128:(c + 1) * 128, 0:SB])
                        else:
                            nc.scalar.dma_start(xtj0[c][:],
                                                xt[c * 128:(c + 1) * 128, 0:SB])
                        if c % 2 == 1:
                            cw = c // 2
                            nc.sync.dma_start(wq_t[cw][:],
                                              wq[cw * 128:(cw + 1) * 128, :])
                    for cw in range(NCHUNK // 2, NCHUNK):
                        nc.sync.dma_start(wq_t[cw][:],
                                          wq[cw * 128:(cw + 1) * 128, :])
                    for c in range(NCHUNK):
                        nc.sync.dma_start(wk_t[c][:], wk[c * 128:(c + 1) * 128, :])
                    for c in range(NCHUNK):
                        nc.gpsimd.dma_start(wv_t[c][:], wv[c * 128:(c + 1) * 128, :])
                    for c in range(NCHUNK):
                        nc.gpsimd.dma_start(wo_t[c][:], wo[c * 128:(c + 1) * 128, :])

                    for j in range(NSB):
                        if j == 0:
                            xtj = xtj0
                        else:
                            xtj = _emit_xt_loads(nc, j, xt, p1x)
                        _p1_block(nc, j, xtj, wq_t, wk_t, wv_t, v_t,
                                  qt_res, kt_res, p1, ps, cst)
                    if 2 in phases:
                        for j in range(NSB):
                            _p2_block(nc, j, v_t, ot_loc, qt_res, kt_res,
                                      p2, ps, cst)
                            if 3 in phases:
                                if fake_gather:
                                    # timing probe: local copy at gather size
                                    for part in range(4):
                                        nc.gpsimd.dma_start(
                                            ot_full[j][part * DC:(part + 1) * DC, :],
                                            ot_loc[j][:])
                                else:
                                    nc.gpsimd.collective_compute(
                                        "AllGather", mybir.AluOpType.bypass,
                                        replica_groups=RG,
                                        ins=[ot_loc[j][:]], outs=[ot_full[j][:]])
                    if 2 not in phases:
                        for h in range(HPC):
                            nc.sync.dma_start(
                                ot_loc[0][h * 128:(h + 1) * 128, :],
                                qt_res[h][:, 0:SB])
                            nc.sync.dma_start(
                                ot_loc[1][h * 128:(h + 1) * 128, :],
                                kt_res[h][:, 0:SB])
                            nc.sync.dma_start(
                                ot_loc[1][h * 128:(h + 1) * 128, 0:DC],
                                v_t[h][:])

                    if 2 in phases and 3 in phases:
                        _phase3(nc, tc, rep, ot_full, wo_t, y, p2, ps)

    nc.compile()
    return nc


def _emit_xt_loads(nc, j, xt, p1x):
    """Stream x^T chunk slices for s-block j (bf16, 16 x [128, 512])."""
    s0 = j * SB
    xtj = [p1x.tile([128, SB], BF16, tag=f"xt{c}", name=f"xt_{j}_{c}")
           for c in range(NCHUNK)]
    for c in range(NCHUNK):
        q = nc.scalar if c % 2 else nc.sync
        q.dma_start(xtj[c][:], xt[c * 128:(c + 1) * 128, s0:s0 + SB])
    return xtj


def _p1_block(nc, j, xtj, wq_t, wk_t, wv_t, v_t, qt_res, kt_res, p1, ps, cst):
    """Projection + RoPE for s-block j: qt/kt slices [j*SB,(j+1)*SB), v blocks."""
    s0 = j * SB
    # q-pass then k-pass: chunk-outer, 4 held accumulators
    for (w_t, res_list, wtag) in ((wq_t, qt_res, "wq"), (wk_t, kt_res, "wk")):
        prj = [ps.tile([128, SB], F32, tag=f"acc{wtag}{h}", bufs=1,
                       name=f"prj{wtag}{j}_{h}") for h in range(HPC)]
        for c in range(NCHUNK):
            for h in range(HPC):
                nc.tensor.matmul(
                    prj[h][:], w_t[c][:, h * 128:(h + 1) * 128],
                    xtj[c][:],
                    start=(c == 0), stop=(c == NCHUNK - 1))
        for h in range(HPC):
            raw = p1.tile([128, SB], BF16, tag="rraw", bufs=1)
            nc.scalar.copy(raw[:], prj[h][:])
            swp = p1.tile([128, SB], BF16, tag="rswp", bufs=1)
            nc.gpsimd.dma_start(swp[0:64, :], raw[64:128, :])
            nc.gpsimd.dma_start(swp[64:128, :], raw[0:64, :])
            t1 = p1.tile([128, SB], BF16, tag="rt1", bufs=1)
            nc.vector.tensor_tensor(
                t1[:], raw[:], cst["c2_t"][:, s0:s0 + SB], op=MULT)
            t2 = p1.tile([128, SB], BF16, tag="rt2", bufs=1)
            nc.vector.tensor_tensor(
                t2[:], swp[:], cst["s2n_t"][:, s0:s0 + SB], op=MULT)
            nc.vector.tensor_tensor(
                res_list[h][:, s0:s0 + SB], t1[:], t2[:], op=ADD)

    # v-pass (natural layout), chunk-outer, into resident v_t
    vps = [ps.tile([128, DC], F32, tag=f"accwq{ss}", bufs=1,
                   name=f"vps{j}_{ss}") for ss in range(SB // 128)]
    for c in range(NCHUNK):
        for ss in range(SB // 128):
            nc.tensor.matmul(
                vps[ss][:], xtj[c][:, ss * 128:(ss + 1) * 128],
                wv_t[c][:], start=(c == 0), stop=(c == NCHUNK - 1))
    for ss in range(SB // 128):
        nc.scalar.copy(v_t[j * 4 + ss][:], vps[ss][:])


def _p2_block(nc, j, v_t, ot_loc, qt_res, kt_res, p2, ps, cst):
    """Causal attention for q-block j (keys/values blocks 0..4j+3)."""
    q0 = j * SB
    nkb = 4 * (j + 1)
    # the reciprocal/mult drain of pair hp is emitted after pair hp+1's
    # first two kb iterations, so hp+1's tri-adds aren't stuck behind it
    # in the in-order DVE queue (they gate exp -> PSUM banks -> PE).
    prev_drain = None
    for hp in range(HPC // 2):
        heads = (2 * hp, 2 * hp + 1)
        pv = {h: ps.tile([128, SB], F32, tag=f"accwk{h % 2}",
                         name=f"pv{j}_{h}") for h in heads}
        dn = {h: ps.tile([128, SB], F32, tag=f"accwk{2 + h % 2}",
                         name=f"dn{j}_{h}") for h in heads}
        # dn/pv emission lags sc/exp by LAG kb iterations so the in-order
        # PE has score matmuls queued ahead of the pv/dn PSUM-bank wait
        # (banks drain through the previous pair's reciprocal/mult chain).
        LAG = 3
        pend = []

        def flush_one():
            h, kb, lo, ep = pend.pop(0)
            nc.tensor.matmul(
                dn[h][:, lo:], cst["ones_t"][:], ep[:, lo:],
                start=(kb == 0), stop=(kb == nkb - 1),
                skip_group_check=True)
            nc.tensor.matmul(
                pv[h][:, lo:], v_t[kb][:, h * 128:(h + 1) * 128],
                ep[:, lo:],
                start=(kb == 0), stop=(kb == nkb - 1),
                skip_group_check=True)

        for kb in range(nkb):
            r = kb - 4 * j
            lo = 0 if r < 0 else r * 128
            for h in heads:
                sc = ps.tile([128, SB], F32, tag=f"accwq{(kb * 2 + h) % 4}",
                             name=f"sc{j}_{h}_{kb}")
                nc.tensor.matmul(
                    sc[:, lo:], kt_res[h][:, kb * 128:(kb + 1) * 128],
                    qt_res[h][:, q0 + lo:q0 + SB],
                    start=True, stop=True)
                if r >= 0:
                    nc.vector.tensor_tensor(
                        sc[:, r * 128:(r + 1) * 128],
                        sc[:, r * 128:(r + 1) * 128], cst["tri_t"][:], op=ADD)
                ep = p2.tile([128, SB], BF16, tag="ep", bufs=10)
                nc.scalar.activation(ep[:, lo:], sc[:, lo:], EXPF, scale=SCALE)
                pend.append((h, kb, lo, ep))
            if kb == 1 and prev_drain is not None:
                prev_drain()
                prev_drain = None
            while len(pend) > 2 * LAG:
                flush_one()
        if prev_drain is not None:
            prev_drain()
            prev_drain = None
        while pend:
            flush_one()

        def mk_drain(pv=pv, dn=dn, heads=heads):
            def drain():
                for h in heads:
                    rec = p2.tile([128, SB], F32, tag="rec", bufs=2,
                                  name=f"rec{j}_{h}")
                    nc.vector.reciprocal(rec[:], dn[h][:])
                    ot = p2.tile([128, SB], BF16, tag="ot", bufs=2,
                                 name=f"ot{j}_{h}")
                    nc.vector.tensor_tensor(ot[:], pv[h][:], rec[:], op=MULT)
                    nc.sync.dma_start(
                        ot_loc[j][h * 128:(h + 1) * 128, :], ot[:])
            return drain

        prev_drain = mk_drain()
    prev_drain()


def _phase3(nc, tc, rep, ot_full, wo_t, y, p3, ps):
    """Column-parallel output projection from gathered o^T halves."""
    for sq in range(4):
        o0 = sq * 512
        otf = [p3.tile([128, 512], BF16, tag=f"otf{c}", bufs=1,
                       name=f"otf{c}_{sq}") for c in range(NCHUNK)]
        for c in range(NCHUNK):
            q = nc.scalar if c % 2 else nc.sync
            q.dma_start(
                otf[c][:],
                ot_full[sq][c * 128:(c + 1) * 128, :])
        for ss in range(4):
            yps = ps.tile([128, DC], F32, tag=f"accwq{ss % 2}", name=f"yps{sq}_{ss}")
            for c in range(NCHUNK):
                nc.tensor.matmul(
                    yps[:], otf[c][:, ss * 128:(ss + 1) * 128],
                    wo_t[c][:], start=(c == 0), stop=(c == NCHUNK - 1))
            ysb = p3.tile([128, DC], F32, tag="ysb", bufs=2)
            nc.scalar.copy(ysb[:], yps[:])
            nc.sync.dma_start(
                y[o0 + ss * 128:o0 + (ss + 1) * 128, :], ysb[:])


_PERM = np.concatenate([np.arange(0, 128, 2), np.arange(1, 128, 2)])
_BF16 = ml_dtypes.bfloat16


def make_in_maps(x, wq, wk, wv, wo, freqs_cos, freqs_sin):
    """Host-side sharding/prep. Returns list of 8 per-core input dicts."""
    cosT = np.ascontiguousarray(freqs_cos.T.astype(np.float32))   # [64, S]
    sinT = np.ascontiguousarray(freqs_sin.T.astype(np.float32))
    c2 = np.concatenate([cosT, cosT], axis=0).astype(_BF16)       # [128, S]
    s2n = np.concatenate([-sinT, sinT], axis=0).astype(_BF16)
    tri = np.where(np.arange(128)[None, :] >= np.arange(128)[:, None],
                   0.0, NEG).astype(np.float32)                   # [k, q]
    ones = np.ones((128, 128), dtype=_BF16)

    xts = [np.ascontiguousarray(x[b].T).astype(_BF16) for b in range(B)]
    in_maps = []
    for c in range(8):
        b, g = divmod(c, 4)
        cols = slice(g * DC, (g + 1) * DC)
        wq_c = np.ascontiguousarray(wq[:, cols]).copy()
        wk_c = np.ascontiguousarray(wk[:, cols]).copy()
        for h in range(HPC):
            blk = slice(h * 128, (h + 1) * 128)
            wq_c[:, blk] = wq_c[:, blk][:, _PERM]
            wk_c[:, blk] = wk_c[:, blk][:, _PERM]
        in_maps.append({
            "xt": xts[b],
            "wq": wq_c.astype(_BF16),
            "wk": wk_c.astype(_BF16),
            "wv": np.ascontiguousarray(wv[:, cols]).astype(_BF16),
            "wo": np.ascontiguousarray(wo[:, cols]).astype(_BF16),
            "c2": c2, "s2n": s2n, "tri": tri, "ones": ones,
        })
    return in_maps


def assemble(results):
    """Concatenate per-core column outputs into [B, S, D]."""
    out = np.empty((B, S, D), dtype=np.float32)
    for c in range(8):
        b, g = divmod(c, 4)
        out[b][:, g * DC:(g + 1) * DC] = results[c]["y"]
    return out


_NC = None


def kernel(x, wq, wk, wv, wo, freqs_cos, freqs_sin):
    global _NC
    x = np.asarray(x); wq = np.asarray(wq); wk = np.asarray(wk)
    wv = np.asarray(wv); wo = np.asarray(wo)
    freqs_cos = np.asarray(freqs_cos); freqs_sin = np.asarray(freqs_sin)
    if _NC is None:
        _NC = build_module()
    in_maps = make_in_maps(x, wq, wk, wv, wo, freqs_cos, freqs_sin)
    res = run_bass_kernel_spmd(_NC, in_maps, core_ids=list(range(8)))
    return assemble(res.results)


# revision 32
# speedup vs baseline: 2.4801x; 2.3325x over previous
"""Multi-head causal attention with RoPE on 8 trn2 NeuronCores.

Problem (hardcoded): B=2, S=2048, D=2048, H=16, Hd=128, fp32 in/out.
  q/k/v = x @ wq/wk/wv; RoPE(q,k); causal softmax(q k^T/sqrt(Hd)) @ v; out @ wo.

Sharding: core c = 4*b + g handles batch b, heads [4g, 4g+4).
  - wq/wk/wv column-parallel (512 cols per core); wo column-parallel fed by an
    AllGather of per-core attention outputs o^T inside each batch group of 4.
  - Host-side prep: x is pre-transposed (and cast to bf16) so no PE transposes
    are needed; per-head even/odd column permutation of wq/wk makes RoPE
    partition-aligned in the transposed [head_dim, S] layout.
  - All matmul operands are bf16 (PSUM accumulation stays f32); weights and
    x^T slices are resident/streamed once — total HBM traffic ~30MB/core vs
    ~96MB for the f32 weight-restreaming variant.
  - o^T is AllGathered (bf16) in four S/4 quarters, one after each
    attention q-block, so only the last gather's latency is tail-exposed
    (covered by the first three quarters of the output projection).
  - Phase 2 is software-pipelined: dn/pv matmuls lag sc/exp by 3 kb
    iterations (PSUM-bank drain hiding), and each head-pair's
    reciprocal/mult drain is emitted inside the next pair's kb loop so the
    in-order DVE queue doesn't block the next pair's tri-adds.
  - RoPE elementwise chain runs in bf16 (DVE 2x mode), halving the DVE
    backlog that otherwise delays the first attention block's mask-adds.

Cost-model sim: 319.5 us span, PE busy 305 us (95.6%). Quiet-window HW
matches at ~325-360 us (shared axon device; loaded windows read higher).
"""
import math
import numpy as np
import ml_dtypes

import concourse.bass as bass
import concourse.tile as tile
from concourse import bacc, mybir
from concourse.bass_utils import run_bass_kernel_spmd

F32 = mybir.dt.float32
BF16 = mybir.dt.bfloat16
EXPF = mybir.ActivationFunctionType.Exp
ADD = mybir.AluOpType.add
MULT = mybir.AluOpType.mult

B, S, D = 2, 2048, 2048
H, HD = 16, 128
HPC = 4              # heads per core
DC = HPC * HD        # 512 d_out per core
NCHUNK = D // 128    # 16 contraction chunks
SB = 512             # s-block (projection and q-block granularity)
NSB = S // SB        # 4
SCALE = 1.0 / math.sqrt(HD)
NEG = -1.0e30

RG = [[0, 1, 2, 3], [4, 5, 6, 7]]


def build_module(trace_sim=False, phases=(1, 2, 3), repeat=1, fake_gather=False):
    nc = bacc.Bacc("TRN2", target_bir_lowering=False, debug=False, num_devices=8)

    xt = nc.dram_tensor("xt", [D, S], BF16, kind="ExternalInput").ap()
    wq = nc.dram_tensor("wq", [D, DC], BF16, kind="ExternalInput").ap()
    wk = nc.dram_tensor("wk", [D, DC], BF16, kind="ExternalInput").ap()
    wv = nc.dram_tensor("wv", [D, DC], BF16, kind="ExternalInput").ap()
    wo = nc.dram_tensor("wo", [D, DC], BF16, kind="ExternalInput").ap()
    c2 = nc.dram_tensor("c2", [128, S], BF16, kind="ExternalInput").ap()
    s2n = nc.dram_tensor("s2n", [128, S], BF16, kind="ExternalInput").ap()
    tri = nc.dram_tensor("tri", [128, 128], F32, kind="ExternalInput").ap()
    ones = nc.dram_tensor("ones", [128, 128], BF16, kind="ExternalInput").ap()
    y = nc.dram_tensor("y", [S, DC], F32, kind="ExternalOutput").ap()

    ot_loc = [nc.dram_tensor(f"ot_loc{i}", [DC, SB], BF16) for i in range(NSB)]
    ot_full = [nc.dram_tensor(f"ot_full{i}", [D, SB], BF16) for i in range(NSB)]

    with tile.TileContext(nc, trace_sim=trace_sim) as tc:
        with tc.tile_pool(name="consts", bufs=1) as cpool:
            ones_t = cpool.tile([128, 128], BF16)
            nc.scalar.dma_start(ones_t[:], ones[:])
            tri_t = cpool.tile([128, 128], F32)
            nc.scalar.dma_start(tri_t[:], tri[:])
            c2_t = cpool.tile([128, S], BF16)
            nc.gpsimd.dma_start(c2_t[:], c2[:])
            s2n_t = cpool.tile([128, S], BF16)
            nc.gpsimd.dma_start(s2n_t[:], s2n[:])
            cst = dict(ones_t=ones_t, tri_t=tri_t, c2_t=c2_t, s2n_t=s2n_t)

            for rep in range(repeat):
                with tc.tile_pool(name=f"wres{rep}", bufs=1) as wpool, \
                     tc.tile_pool(name=f"qkres{rep}", bufs=1) as qkpool, \
                     tc.tile_pool(name=f"vres{rep}", bufs=1) as vpool, \
                     tc.tile_pool(name=f"p1sb{rep}", bufs=2) as p1, \
                     tc.tile_pool(name=f"p1xt{rep}", bufs=2) as p1x, \
                     tc.tile_pool(name=f"p2sb{rep}", bufs=3) as p2, \
                     tc.tile_pool(name=f"ps{rep}", bufs=1, space="PSUM") as ps:
                    # resident weights: loaded once, used by all 4 s-blocks.
                    wq_t = [wpool.tile([128, DC], BF16, name=f"wqt{c}")
                            for c in range(NCHUNK)]
                    wk_t = [wpool.tile([128, DC], BF16, name=f"wkt{c}")
                            for c in range(NCHUNK)]
                    wv_t = [wpool.tile([128, DC], BF16, name=f"wvt{c}")
                            for c in range(NCHUNK)]
                    wo_t = [wpool.tile([128, DC], BF16, name=f"wot{c}")
                            for c in range(NCHUNK)]

                    qt_res = [qkpool.tile([128, S], BF16, name=f"qt{h}")
                              for h in range(HPC)]
                    kt_res = [qkpool.tile([128, S], BF16, name=f"kt{h}")
                              for h in range(HPC)]
                    v_t = [vpool.tile([128, DC], BF16, name=f"v{kb}")
                           for kb in range(S // 128)]

                    # DMA issue order follows need order so PE starts ~2us in:
                    # sync: [xt0/wq interleaved, wk], scalar: [consts,
                    # xt block0 odds], gpsimd: [c2/s2n, wv, wo].
                    xtj0 = [p1x.tile([128, SB], BF16, tag=f"xt{c}",
                                     name=f"xt_0_{c}") for c in range(NCHUNK)]
                    for c in range(NCHUNK):
                        if c % 2 == 0:
                            nc.sync.dma_start(xtj0[c][:],
                                              xt[c * 128:(c + 1) * 128, 0:SB])
                        else:
                            nc.scalar.dma_start(xtj0[c][:],
                                                xt[c * 128:(c + 1) * 128, 0:SB])
                        if c % 2 == 1:
                            cw = c // 2
                            nc.sync.dma_start(wq_t[cw][:],
                                              wq[cw * 128:(cw + 1) * 128, :])
                    for cw in range(NCHUNK // 2, NCHUNK):
                        nc.sync.dma_start(wq_t[cw][:],
                                          wq[cw * 128:(cw + 1) * 128, :])
                    for c in range(NCHUNK):
                        nc.sync.dma_start(wk_t[c][:], wk[c * 128:(c + 1) * 128, :])
                    for c in range(NCHUNK):
                        nc.gpsimd.dma_start(wv_t[c][:], wv[c * 128:(c + 1) * 128, :])
                    for c in range(NCHUNK):
                        nc.gpsimd.dma_start(wo_t[c][:], wo[c * 128:(c + 1) * 128, :])

                    for j in range(NSB):
                        if j == 0:
                            xtj = xtj0
                        else:
                            xtj = _emit_xt_loads(nc, j, xt, p1x)
                        _p1_block(nc, j, xtj, wq_t, wk_t, wv_t, v_t,
                                  qt_res, kt_res, p1, ps, cst)
                    if 2 in phases:
                        for j in range(NSB):
                            _p2_block(nc, j, v_t, ot_loc, qt_res, kt_res,
                                      p2, ps, cst)
                            if 3 in phases:
                                if fake_gather:
                                    # timing probe: local copy at gather size
                                    for part in range(4):
                                        nc.gpsimd.dma_start(
                                            ot_full[j][part * DC:(part + 1) * DC, :],
                                            ot_loc[j][:])
                                else:
                                    nc.gpsimd.collective_compute(
                                        "AllGather", mybir.AluOpType.bypass,
                                        replica_groups=RG,
                                        ins=[ot_loc[j][:]], outs=[ot_full[j][:]])
                    if 2 not in phases:
                        for h in range(HPC):
                            nc.sync.dma_start(
                                ot_loc[0][h * 128:(h + 1) * 128, :],
                                qt_res[h][:, 0:SB])
                            nc.sync.dma_start(
                                ot_loc[1][h * 128:(h + 1) * 128, :],
                                kt_res[h][:, 0:SB])
                            nc.sync.dma_start(
                                ot_loc[1][h * 128:(h + 1) * 128, 0:DC],
                                v_t[h][:])

                    if 2 in phases and 3 in phases:
                        _phase3(nc, tc, rep, ot_full, wo_t, y, p2, ps)

    nc.compile()
    return nc


def _emit_xt_loads(nc, j, xt, p1x):
    """Stream x^T chunk slices for s-block j (bf16, 16 x [128, 512])."""
    s0 = j * SB
    xtj = [p1x.tile([128, SB], BF16, tag=f"xt{c}", name=f"xt_{j}_{c}")
           for c in range(NCHUNK)]
    for c in range(NCHUNK):
        q = nc.scalar if c % 2 else nc.sync
        q.dma_start(xtj[c][:], xt[c * 128:(c + 1) * 128, s0:s0 + SB])
    return xtj


def _p1_block(nc, j, xtj, wq_t, wk_t, wv_t, v_t, qt_res, kt_res, p1, ps, cst):
    """Projection + RoPE for s-block j: qt/kt slices [j*SB,(j+1)*SB), v blocks."""
    s0 = j * SB
    # q-pass then k-pass: chunk-outer, 4 held accumulators
    for (w_t, res_list, wtag) in ((wq_t, qt_res, "wq"), (wk_t, kt_res, "wk")):
        prj = [ps.tile([128, SB], F32, tag=f"acc{wtag}{h}", bufs=1,
                       name=f"prj{wtag}{j}_{h}") for h in range(HPC)]
        for c in range(NCHUNK):
            for h in range(HPC):
                nc.tensor.matmul(
                    prj[h][:], w_t[c][:, h * 128:(h + 1) * 128],
                    xtj[c][:],
                    start=(c == 0), stop=(c == NCHUNK - 1))
        for h in range(HPC):
            raw = p1.tile([128, SB], BF16, tag="rraw", bufs=1)
            nc.scalar.copy(raw[:], prj[h][:])
            swp = p1.tile([128, SB], BF16, tag="rswp", bufs=1)
            nc.gpsimd.dma_start(swp[0:64, :], raw[64:128, :])
            nc.gpsimd.dma_start(swp[64:128, :], raw[0:64, :])
            t1 = p1.tile([128, SB], BF16, tag="rt1", bufs=1)
            nc.vector.tensor_tensor(
                t1[:], raw[:], cst["c2_t"][:, s0:s0 + SB], op=MULT)
            t2 = p1.tile([128, SB], BF16, tag="rt2", bufs=1)
            nc.vector.tensor_tensor(
                t2[:], swp[:], cst["s2n_t"][:, s0:s0 + SB], op=MULT)
            nc.vector.tensor_tensor(
                res_list[h][:, s0:s0 + SB], t1[:], t2[:], op=ADD)

    # v-pass (natural layout), chunk-outer, into resident v_t
    vps = [ps.tile([128, DC], F32, tag=f"accwq{ss}", bufs=1,
                   name=f"vps{j}_{ss}") for ss in range(SB // 128)]
    for c in range(NCHUNK):
        for ss in range(SB // 128):
            nc.tensor.matmul(
                vps[ss][:], xtj[c][:, ss * 128:(ss + 1) * 128],
                wv_t[c][:], start=(c == 0), stop=(c == NCHUNK - 1))
    for ss in range(SB // 128):
        nc.scalar.copy(v_t[j * 4 + ss][:], vps[ss][:])


def _p2_block(nc, j, v_t, ot_loc, qt_res, kt_res, p2, ps, cst):
    """Causal attention for q-block j (keys/values blocks 0..4j+3)."""
    q0 = j * SB
    nkb = 4 * (j + 1)
    # the reciprocal/mult drain of pair hp is emitted after pair hp+1's
    # first two kb iterations, so hp+1's tri-adds aren't stuck behind it
    # in the in-order DVE queue (they gate exp -> PSUM banks -> PE).
    prev_drain = None
    for hp in range(HPC // 2):
        heads = (2 * hp, 2 * hp + 1)
        pv = {h: ps.tile([128, SB], F32, tag=f"accwk{h % 2}",
                         name=f"pv{j}_{h}") for h in heads}
        dn = {h: ps.tile([128, SB], F32, tag=f"accwk{2 + h % 2}",
                         name=f"dn{j}_{h}") for h in heads}
        # dn/pv emission lags sc/exp by LAG kb iterations so the in-order
        # PE has score matmuls queued ahead of the pv/dn PSUM-bank wait
        # (banks drain through the previous pair's reciprocal/mult chain).
        LAG = 3
        pend = []

        def flush_one():
            h, kb, lo, ep = pend.pop(0)
            nc.tensor.matmul(
                dn[h][:, lo:], cst["ones_t"][:], ep[:, lo:],
                start=(kb == 0), stop=(kb == nkb - 1),
                skip_group_check=True)
            nc.tensor.matmul(
                pv[h][:, lo:], v_t[kb][:, h * 128:(h + 1) * 128],
                ep[:, lo:],
                start=(kb == 0), stop=(kb == nkb - 1),
                skip_group_check=True)

        for kb in range(nkb):
            r = kb - 4 * j
            lo = 0 if r < 0 else r * 128
            for h in heads:
                sc = ps.tile([128, SB], F32, tag=f"accwq{(kb * 2 + h) % 4}",
                             name=f"sc{j}_{h}_{kb}")
                nc.tensor.matmul(
                    sc[:, lo:], kt_res[h][:, kb * 128:(kb + 1) * 128],
                    qt_res[h][:, q0 + lo:q0 + SB],
                    start=True, stop=True)
                if r >= 0:
                    nc.vector.tensor_tensor(
                        sc[:, r * 128:(r + 1) * 128],
                        sc[:, r * 128:(r + 1) * 128], cst["tri_t"][:], op=ADD)
                ep = p2.tile([128, SB], BF16, tag="ep", bufs=10)
                nc.scalar.activation(ep[:, lo:], sc[:, lo:], EXPF, scale=SCALE)
                pend.append((h, kb, lo, ep))
            if kb == 1 and prev_drain is not None:
                prev_drain()
                prev_drain = None
            while len(pend) > 2 * LAG:
                flush_one()
        if prev_drain is not None:
            prev_drain()
            prev_drain = None
        while pend:
            flush_one()

        def mk_drain(pv=pv, dn=dn, heads=heads):
            def drain():
                for h in heads:
                    rec = p2.tile([128, SB], F32, tag="rec", bufs=2,
                                  name=f"rec{j}_{h}")
                    nc.vector.reciprocal(rec[:], dn[h][:])
                    ot = p2.tile([128, SB], BF16, tag="ot", bufs=2,
                                 name=f"ot{j}_{h}")
                    nc.vector.tensor_tensor(ot[:], pv[h][:], rec[:], op=MULT)
                    nc.sync.dma_start(
                        ot_loc[j][h * 128:(h + 1) * 128, :], ot[:])
            return drain

        prev_drain = mk_drain()
    prev_drain()


def _phase3(nc, tc, rep, ot_full, wo_t, y, p3, ps):
    """Column-parallel output projection from gathered o^T halves."""
    for sq in range(4):
        o0 = sq * 512
        otf = [p3.tile([128, 512], BF16, tag=f"otf{c}", bufs=1,
                       name=f"otf{c}_{sq}") for c in range(NCHUNK)]
        for c in range(NCHUNK):
            q = nc.scalar if c % 2 else nc.sync
            q.dma_start(
                otf[c][:],
                ot_full[sq][c * 128:(c + 1) * 128, :])
        for ss in range(4):
            yps = ps.tile([128, DC], F32, tag=f"accwq{ss % 2}", name=f"yps{sq}_{ss}")
            for c in range(NCHUNK):
                nc.tensor.matmul(
                    yps[:], otf[c][:, ss * 128:(ss + 1) * 128],
                    wo_t[c][:], start=(c == 0), stop=(c == NCHUNK - 1))
            ysb = p3.tile([128, DC], F32, tag="ysb", bufs=2)
            nc.scalar.copy(ysb[:], yps[:])
            nc.sync.dma_start(
                y[o0 + ss * 128:o0 + (ss + 1) * 128, :], ysb[:])


_PERM = np.concatenate([np.arange(0, 128, 2), np.arange(1, 128, 2)])
_BF16 = ml_dtypes.bfloat16


def make_in_maps(x, wq, wk, wv, wo, freqs_cos, freqs_sin):
    """Host-side sharding/prep. Returns list of 8 per-core input dicts."""
    cosT = np.ascontiguousarray(freqs_cos.T.astype(np.float32))   # [64, S]
    sinT = np.ascontiguousarray(freqs_sin.T.astype(np.float32))
    c2 = np.concatenate([cosT, cosT], axis=0).astype(_BF16)       # [128, S]
    s2n = np.concatenate([-sinT, sinT], axis=0).astype(_BF16)
    tri = np.where(np.arange(128)[None, :] >= np.arange(128)[:, None],
                   0.0, NEG).astype(np.float32)                   # [k, q]
    ones = np.ones((128, 128), dtype=_BF16)

    xts = [np.ascontiguousarray(x[b].T).astype(_BF16) for b in range(B)]
    in_maps = []
    for c in range(8):
        b, g = divmod(c, 4)
        cols = slice(g * DC, (g + 1) * DC)
        wq_c = np.ascontiguousarray(wq[:, cols]).copy()
        wk_c = np.ascontiguousarray(wk[:, cols]).copy()
        for h in range(HPC):
            blk = slice(h * 128, (h + 1) * 128)
            wq_c[:, blk] = wq_c[:, blk][:, _PERM]
            wk_c[:, blk] = wk_c[:, blk][:, _PERM]
        in_maps.append({
            "xt": xts[b],
            "wq": wq_c.astype(_BF16),
            "wk": wk_c.astype(_BF16),
            "wv": np.ascontiguousarray(wv[:, cols]).astype(_BF16),
            "wo": np.ascontiguousarray(wo[:, cols]).astype(_BF16),
            "c2": c2, "s2n": s2n, "tri": tri, "ones": ones,
        })
    return in_maps


def assemble(results):
    """Concatenate per-core column outputs into [B, S, D]."""
    out = np.empty((B, S, D), dtype=np.float32)
    for c in range(8):
        b, g = divmod(c, 4)
        out[b][:, g * DC:(g + 1) * DC] = results[c]["y"]
    return out


_NC = None


def kernel(x, wq, wk, wv, wo, freqs_cos, freqs_sin):
    global _NC
    x = np.asarray(x); wq = np.asarray(wq); wk = np.asarray(wk)
    wv = np.asarray(wv); wo = np.asarray(wo)
    freqs_cos = np.asarray(freqs_cos); freqs_sin = np.asarray(freqs_sin)
    if _NC is None:
        _NC = build_module()
    in_maps = make_in_maps(x, wq, wk, wv, wo, freqs_cos, freqs_sin)
    res = run_bass_kernel_spmd(_NC, in_maps, core_ids=list(range(8)))
    return assemble(res.results)
